# revision 1
# baseline (speedup 1.0000x reference)
"""Single-head attention (B=4, S=4096, E=1024, D=64) on 8 TRN2 NeuronCores.

Sharding: data-parallel over (batch, query-half): core c handles batch
b = c // 2 and query rows [h*2048, (h+1)*2048) with h = c % 2. Each core
computes Q for its own 2048 rows and K/V for the full 4096 rows of its batch
(inputs are shipped host-pretransposed per half, so no duplicated DMA).

Per-core dataflow (TensorE matmuls in bf16 — fp32/fp32r matmuls run the PE
at half clock; fp32 accumulation in PSUM). Projections pack TWO weight
matrices into one 128-wide stationary operand:
  qk [128, 2048] = [K^T_own; Q^T_own]     (pass A, lhsT = [WkT | WqT])
  kv [128, 2048] = [K^T_oth; V^T_oth]     (pass B, lhsT = [WkT | WvT])
  vt [65, 2048]  = V^T_own + ones row      (pass C, lhsT = WvT)
Q^T / V^T_oth are then shifted to base partition 0 by SBUF-to-SBUF DMAs
(matmul operands must share a base partition).
  scores^T[k, q] = K^T.T @ Q^T -> exp -> P bf16
  attn^T[65, q] += V_aug.T @ P   (row 64 accumulates softmax denominators)
  output = attn^T with denominators; host transposes + normalizes.

The exp is split across two engines so ScalarE (1 elem/cycle/lane at
1.2 GHz, ~1.15us per [128,1024] tile) stops pacing the pipeline: 2/3 of
k-tiles get the exact ACT exp on ScalarE; every third tile is computed on
VectorE with a one-instruction Schraudolph bit-trick: i16 = rne(x*A + B)
reinterpreted as bf16 approximates exp(SCALE*x) (piecewise-linear mantissa
chord, ~1.8% log-error sawtooth whose mean bias cancels in the softmax
numerator/denominator ratio; applied to 20/64 of the weights it adds
~0.6% output rel err). The two engines use SEPARATE P-tile pools — a
shared pool serializes them via buffer-reuse ordering.

The attention runs as TWO passes over q (1024 columns each): the attn
accumulator then fits 2 PSUM banks, freeing a third scores slot (PSUM slot
contention paced the single-pass version), and pass 0's output ships
mid-kernel.

The HAM duty controller halves the PE clock after ~2.5us of PE idleness
and takes 5-13us at half clock to re-grant full duty, so the kernel keeps
the PE streaming: junk-fed 512-col warm-up matmuls run from the instant
the PE preamble ends until the first input DMAs land (gated only on a
VectorE memset, not on make_identity's gpsimd iota), and junk fillers
bridge the group-2 DMA wait at pass-0 iters 4-5. Input DMA is issued in
deadline order (wt + own q-cols 0:1024, then own 1024:2048, then the
other half) across the sync/scalar/gpsimd queues; the pass-0 side-slot
schedule (projection lumps A2/C2/A3/C3, then B0-B3) tracks the measured
arrival of those groups.
"""

import numpy as np

B, S, E, D = 4, 4096, 1024, 64
HALF = S // 2
N_CORES = 8
SCALE = 1.0 / np.sqrt(D)

NE = E // 128  # 8 e-tiles
NKT = S // 128  # 32 k-tiles
N_WARM = 20  # 512-col PE warm-up matmuls covering the preamble + DMA wait

# Schraudolph exp-approx constants (bf16 bit pattern via int16):
#   i16 = round(x * A16 + B16); bitcast bf16 ~= exp(SCALE * x)
LOG2E = 1.4426950408889634
A16 = SCALE * 128.0 * LOG2E
B16 = 127.0 * 128.0 - 7.3


def _exp_eng(k):
    """Which engine computes exp for k-tile `k` (0..NKT-1) of a pass.

    'S' = ScalarE exact ACT exp; 'V' = VectorE Schraudolph. (GpSimd
    cannot read PSUM, so it can't help here.) 22 exact + 10 approx per
    pass: the approx noise (~1.8% per weight) lands on 20/64 of the
    softmax weights -> ~0.6% output rel err. A 50/50 split was tried and
    measured slower: VectorE's other duties (transpose copies, output
    copies) make it the pole once it carries half the exps.
    """
    return "V" if k % 3 == 2 else "S"

_CACHE = {}


def _build():
    if "nc" in _CACHE:
        return _CACHE["nc"]

    from contextlib import ExitStack

    import concourse.bacc as bacc
    import concourse.tile as tile
    from concourse import mybir
    from concourse.masks import make_identity

    FP32 = mybir.dt.float32
    BF16 = mybir.dt.bfloat16
    I16 = mybir.dt.int16
    Exp = mybir.ActivationFunctionType.Exp
    Mult = mybir.AluOpType.mult
    Add = mybir.AluOpType.add

    nc = bacc.Bacc(
        "TRN2", target_bir_lowering=False, debug=False, num_devices=N_CORES
    )

    xt_q_d = nc.dram_tensor("xt_q", [E, HALF], BF16, kind="ExternalInput").ap()
    xt_o_d = nc.dram_tensor("xt_o", [E, HALF], BF16, kind="ExternalInput").ap()
    wt_d = nc.dram_tensor("wt", [E, 320], BF16, kind="ExternalInput").ap()
    out_d = nc.dram_tensor("out", [D + 1, HALF], FP32, kind="ExternalOutput").ap()

    with tile.TileContext(nc) as tc, ExitStack() as ctx:
        const = ctx.enter_context(tc.tile_pool(name="const", bufs=1))
        big = ctx.enter_context(tc.tile_pool(name="big", bufs=1))
        # separate P pools per exp engine: a shared pool serializes the
        # scalar and vector exps against each other via slot-reuse ordering
        pps = ctx.enter_context(tc.tile_pool(name="pps", bufs=4))
        ppv = ctx.enter_context(tc.tile_pool(name="ppv", bufs=3))
        psA = ctx.enter_context(tc.tile_pool(name="psA", bufs=3, space="PSUM"))
        psB = ctx.enter_context(tc.tile_pool(name="psB", bufs=1, space="PSUM"))

        identB = const.tile([128, 128], BF16)
        junk = const.tile([128, 512], BF16)
        # memset on vector: ready right at the end of vector's preamble, so
        # the PE warm-up can start ~2.5us before make_identity's gpsimd
        # iota would allow.
        nc.vector.memset(junk[:, :], 0.0)
        make_identity(nc, identB)

        xt = big.tile([128, NE, S], BF16)  # x^T; cols [0, HALF) = own q-rows
        wt = big.tile([128, NE, 320], BF16)  # [WkT|WqT | WkT|WvT | WvT]
        qk = big.tile([128, HALF], BF16)  # rows 0-63 K^T own, 64-127 Q^T own
        kv = big.tile([128, HALF], BF16)  # rows 0-63 K^T oth, 64-127 V^T oth
        qts = big.tile([64, HALF], BF16)  # Q^T shifted to base partition 0
        vto = big.tile([64, HALF], BF16)  # V^T other shifted to base part. 0
        vt = big.tile([65, HALF], BF16)  # V^T own; row 64 = ones
        vn = big.tile([128, NKT, D + 1], BF16)  # V natural + ones column
        att_sb = big.tile([65, HALF], FP32)  # attn^T + denominator row

        # --- PE warm-up: the HAM duty controller halves the PE clock after
        # ~2.5us of idleness and takes 5-13us at half clock to restore full
        # duty, so keep the PE streaming junk matmuls from the instant its
        # preamble ends until the first input DMAs land.
        warm = psA.tile([128, 1024], FP32, tag="ps")
        for _ in range(N_WARM):
            nc.tensor.matmul(
                out=warm[0:128, 0:512],
                lhsT=junk[:, 0:128],
                rhs=junk[:, :],
                start=True,
                stop=True,
            )

        # --- input DMAs. One dma_start ~= one DMA queue, so split per
        # e-tile. Priority: wt, own cols 0:1024 (gates pass A / first exp),
        # then own cols 1024:2048 (A2/A3 lumps, ~iter 5+), then the other
        # half (B lumps, ~iter 13+). sync stays free-ish for the shifts.
        nc.sync.dma_start(out=wt[:, :, :], in_=wt_d.rearrange("(t p) d -> p t d", p=128))
        # cols 0:1024 (gates the first exp): spread across all three
        # DMA-capable engines; gpsimd's SWDGE has multiple queues.
        first_engs = [nc.gpsimd, nc.sync, nc.gpsimd, nc.scalar,
                      nc.gpsimd, nc.sync, nc.gpsimd, nc.scalar]
        for et in range(NE):
            first_engs[et].dma_start(
                out=xt[:, et, 0:1024],
                in_=xt_q_d[et * 128 : (et + 1) * 128, 0:1024],
            )
        for et in range(NE):
            nc.gpsimd.dma_start(
                out=xt[:, et, 1024:2048],
                in_=xt_q_d[et * 128 : (et + 1) * 128, 1024:2048],
            )
        for et in range(NE):
            eng = [nc.gpsimd, nc.sync][et % 2]
            eng.dma_start(
                out=xt[:, et, HALF:S],
                in_=xt_o_d[et * 128 : (et + 1) * 128, :],
            )

        nc.vector.memset(vt[64:65, :], 1.0)

        # one packed projection half-chunk of 512 cols
        def proj_half(w0, wm, dst, src_x0, d0):
            acc = psA.tile([128, 1024], FP32, tag="ps")
            for et in range(NE):
                nc.tensor.matmul(
                    out=acc[0:wm, 0:512],
                    lhsT=wt[:, et, w0 : w0 + wm],
                    rhs=xt[:, et, src_x0 : src_x0 + 512],
                    start=(et == 0),
                    stop=(et == NE - 1),
                )
            nc.vector.tensor_copy(out=dst[:, d0 : d0 + 512], in_=acc[0:wm, 0:512])

        def shift(dst, src, d0):
            # scalar's DMA queue is otherwise idle until the exps begin,
            # so the shifts never wait behind bulk input pieces.
            nc.scalar.dma_start(
                out=dst[:, d0 : d0 + 512], in_=src[64:128, d0 : d0 + 512]
            )

        def v_transpose(k):
            tp = psA.tile([128, 1024], BF16, tag="ps")
            if k < 16:  # own half: vt carries the ones row
                nc.tensor.transpose(
                    out=tp[0:128, 0:65],
                    in_=vt[:, k * 128 : (k + 1) * 128],
                    identity=identB[0:65, 0:65],
                )
                nc.vector.tensor_copy(out=vn[:, k, :], in_=tp[0:128, 0:65])
            else:  # other half: V^T shifted into vto (base partition 0)
                j = k - 16
                nc.tensor.transpose(
                    out=tp[0:128, 0:64],
                    in_=vto[:, j * 128 : (j + 1) * 128],
                    identity=identB[0:64, 0:64],
                )
                nc.vector.memset(vn[:, k, D : D + 1], 1.0)
                nc.vector.tensor_copy(out=vn[:, k, 0:D], in_=tp[0:128, 0:64])

        # --- prologue: pass A halves 0-1 (K^T + Q^T own, q-cols 0:1024) ---
        for hh in range(2):
            proj_half(0, 128, qk, hh * 512, hh * 512)
            shift(qts, qk, hh * 512)

        # side-slot schedule for pass 0: iter k -> (kind, half-index).
        # Slots follow the input-DMA arrival order: own cols 0:1024 first
        # (C0/C1), then own 1024:2048 (~iter 5-6: A2/C2/A3/C3), then the
        # other half (~iter 12-13: B lumps).
        SIDE = {
            1: ("C", 0), 3: ("C", 1),
            6: ("A", 2), 7: ("C", 2), 10: ("A", 3), 11: ("C", 3),
            14: ("B", 0), 16: ("B", 1), 18: ("B", 2), 20: ("B", 3),
        }

        def side_work(k):
            s = SIDE.get(k)
            if s is not None:
                kind, hh = s
                if kind == "A":
                    proj_half(0, 128, qk, hh * 512, hh * 512)
                    shift(qts, qk, hh * 512)
                elif kind == "C":
                    proj_half(256, 64, vt[0:64, :], hh * 512, hh * 512)
                else:
                    proj_half(128, 128, kv, HALF + hh * 512, hh * 512)
                    shift(vto, kv, hh * 512)
            if k == 2:
                v_transpose(0)
                v_transpose(1)
            elif k >= 3:
                v_transpose(k - 1)
                if k == NKT - 1:
                    v_transpose(NKT - 1)

        out_engs = [nc.sync, nc.gpsimd]

        # --- two q-passes of 1024 columns each ---
        for ps in range(2):
            att_ps = psB.tile([128, 1024], FP32)
            p_tiles = {}

            for k in range(NKT):
                if k < 16:
                    klhs = qk[0:64, k * 128 : (k + 1) * 128]
                else:
                    klhs = kv[0:64, (k - 16) * 128 : (k - 15) * 128]

                sc = psA.tile([128, 1024], FP32, tag="ps")
                if ps == 0 and k in (4, 5):
                    # DMA-wait fillers: keep the PE streaming while the
                    # group-2 input pieces land (overwritten by the
                    # start=True scores matmuls below).
                    for _ in range(4):
                        nc.tensor.matmul(
                            out=sc[:, 0:512],
                            lhsT=junk[:, 0:128],
                            rhs=junk[:, :],
                            start=True,
                            stop=True,
                        )
                for c in range(2):
                    q0 = ps * 1024 + c * 512
                    nc.tensor.matmul(
                        out=sc[:, c * 512 : (c + 1) * 512],
                        lhsT=klhs,
                        rhs=qts[:, q0 : q0 + 512],
                        start=True,
                        stop=True,
                    )
                eng = _exp_eng(k)
                if eng == "S":
                    p = pps.tile([128, 1024], BF16)
                    nc.scalar.activation(
                        out=p[:, :], in_=sc[:, :], func=Exp, scale=SCALE
                    )
                else:
                    p = ppv.tile([128, 1024], BF16)
                    nc.vector.tensor_scalar(
                        p[:, :].bitcast(I16), sc[:, :], A16, B16, Mult, Add
                    )
                p_tiles[k] = p

                if ps == 0:
                    side_work(k)
                # attn lags THREE iters behind scores: at lag 2 the exp
                # chain (ScalarE ~1.15us, 2 of every 3 tiles) misses the
                # deadline by ~230ns every iteration and paces the whole
                # pipeline at 1085ns/iter instead of the PE-bound 853ns.
                if k >= 3:
                    _attn(nc, att_ps, vn, p_tiles, k - 3)

            for kt in (NKT - 3, NKT - 2, NKT - 1):
                _attn(nc, att_ps, vn, p_tiles, kt)

            # ship this pass's attn^T + denominators (host normalizes)
            for c in range(2):
                cols = slice(ps * 1024 + c * 512, ps * 1024 + (c + 1) * 512)
                pcols = slice(c * 512, (c + 1) * 512)
                nc.vector.tensor_copy(out=att_sb[:, cols], in_=att_ps[0:65, pcols])
                out_engs[c].dma_start(out=out_d[:, cols], in_=att_sb[:, cols])

    nc.compile()
    _CACHE["nc"] = nc
    return nc


def _attn(nc, att_ps, vn, p_tiles, k):
    p = p_tiles.pop(k)
    for c in range(2):
        nc.tensor.matmul(
            out=att_ps[0:65, c * 512 : (c + 1) * 512],
            lhsT=vn[:, k, :],
            rhs=p[:, c * 512 : (c + 1) * 512],
            start=(k == 0),
            stop=(k == NKT - 1),
            skip_group_check=True,
        )


def _make_in_maps(x, Wq, Wk, Wv):
    import ml_dtypes

    bf16 = ml_dtypes.bfloat16
    xT = np.ascontiguousarray(x.transpose(0, 2, 1)).astype(bf16)  # [B, E, S]
    wt = np.concatenate(
        [Wk.T, Wq.T, Wk.T, Wv.T, Wv.T], axis=1
    ).astype(bf16)  # [E, 320]
    in_maps = []
    for c in range(N_CORES):
        b, h = divmod(c, 2)
        in_maps.append(
            {
                "xt_q": np.ascontiguousarray(xT[b, :, h * HALF : (h + 1) * HALF]),
                "xt_o": np.ascontiguousarray(
                    xT[b, :, (1 - h) * HALF : (2 - h) * HALF]
                ),
                "wt": wt,
            }
        )
    return in_maps


def _run(x, Wq, Wk, Wv, trace=False):
    from concourse.bass_utils import run_bass_kernel_spmd

    nc = _build()
    in_maps = _make_in_maps(x, Wq, Wk, Wv)
    res = run_bass_kernel_spmd(
        nc, in_maps, core_ids=list(range(N_CORES)), trace=trace
    )
    out = np.empty((B, S, D), dtype=np.float32)
    for c in range(N_CORES):
        b, h = divmod(c, 2)
        att = res.results[c]["out"]  # [65, HALF]: attn^T rows + denom row
        out[b, h * HALF : (h + 1) * HALF, :] = (att[0:D] / att[D : D + 1]).T
    return out, res


def kernel(x, Wq, Wk, Wv):
    out, _ = _run(
        np.asarray(x, dtype=np.float32),
        np.asarray(Wq, dtype=np.float32),
        np.asarray(Wk, dtype=np.float32),
        np.asarray(Wv, dtype=np.float32),
    )
    return out



# revision 6
# speedup vs baseline: 1.0674x; 1.0674x over previous
"""Single-head attention (B=4, S=4096, E=1024, D=64) on 8 TRN2 NeuronCores.

Sharding: data-parallel over (batch, query-half): core c handles batch
b = c // 2 and query rows [h*2048, (h+1)*2048) with h = c % 2. Each core
computes Q for its own 2048 rows and K/V for the full 4096 rows of its batch
(inputs are shipped host-pretransposed per half, so no duplicated DMA).

Per-core dataflow (TensorE matmuls in bf16 — fp32/fp32r matmuls run the PE
at half clock; fp32 accumulation in PSUM). Projections pack TWO weight
matrices into one 128-wide stationary operand:
  qk [128, 2048] = [K^T_own; Q^T_own]     (pass A, lhsT = [WkT | WqT])
  kv [128, 2048] = [K^T_oth; V^T_oth]     (pass B, lhsT = [WkT | WvT])
  vt [65, 2048]  = V^T_own + ones row      (pass C, lhsT = WvT)
Q^T / V^T_oth are then shifted to base partition 0 by SBUF-to-SBUF DMAs
(matmul operands must share a base partition).
  scores^T[k, q] = K^T.T @ Q^T -> exp -> P bf16
  attn^T[65, q] += V_aug.T @ P   (row 64 accumulates softmax denominators)
  output = attn^T with denominators; host transposes + normalizes.

The exp is split across two engines so ScalarE (1 elem/cycle/lane at
1.2 GHz, ~1.15us per [128,1024] tile) stops pacing the pipeline: 2/3 of
k-tiles get the exact ACT exp on ScalarE; every third tile is computed on
VectorE with a one-instruction Schraudolph bit-trick: i16 = rne(x*A + B)
reinterpreted as bf16 approximates exp(SCALE*x) (piecewise-linear mantissa
chord, ~1.8% log-error sawtooth whose mean bias cancels in the softmax
numerator/denominator ratio; applied to 20/64 of the weights it adds
~0.6% output rel err). The two engines use SEPARATE P-tile pools — a
shared pool serializes them via buffer-reuse ordering.

The attention runs as TWO passes over q (1024 columns each): the attn
accumulator then fits 2 PSUM banks, freeing a third scores slot (PSUM slot
contention paced the single-pass version), and pass 0's output ships
mid-kernel.

The HAM duty controller halves the PE clock after ~2.5us of PE idleness
and takes 5-13us at half clock to re-grant full duty, so the kernel keeps
the PE streaming: junk-fed 512-col warm-up matmuls run from the instant
the PE preamble ends until the first input DMAs land (gated only on a
VectorE memset, not on make_identity's gpsimd iota), and junk fillers
bridge the group-2 DMA wait at pass-0 iters 4-5. Input DMA is issued in
deadline order (wt + own q-cols 0:1024, then own 1024:2048, then the
other half) across the sync/scalar/gpsimd queues; the pass-0 side-slot
schedule (projection lumps A2/C2/A3/C3, then B0-B3) tracks the measured
arrival of those groups.
"""

import numpy as np

B, S, E, D = 4, 4096, 1024, 64
HALF = S // 2
N_CORES = 8
SCALE = 1.0 / np.sqrt(D)

NE = E // 128  # 8 e-tiles
NKT = S // 128  # 32 k-tiles
N_WARM = 16  # 512-col PE warm-up matmuls covering the preamble + DMA wait

# Schraudolph exp-approx constants (bf16 bit pattern via int16):
#   i16 = round(x * A16 + B16); bitcast bf16 ~= exp(SCALE * x)
LOG2E = 1.4426950408889634
A16 = SCALE * 128.0 * LOG2E
B16 = 127.0 * 128.0 - 7.3


def _exp_eng(k):
    """Which engine computes exp for k-tile `k` (0..NKT-1) of a pass.

    'S' = ScalarE exact ACT exp; 'V' = VectorE Schraudolph. (GpSimd
    cannot read PSUM, so it can't help here.) 22 exact + 10 approx per
    pass: the approx noise (~1.8% per weight) lands on 20/64 of the
    softmax weights -> ~0.6% output rel err. A 50/50 split was tried and
    measured slower: VectorE's other duties (transpose copies, output
    copies) make it the pole once it carries half the exps.
    """
    return "V" if k % 3 == 2 else "S"

_CACHE = {}


def _build():
    if "nc" in _CACHE:
        return _CACHE["nc"]

    from contextlib import ExitStack

    import concourse.bacc as bacc
    import concourse.tile as tile
    from concourse import mybir
    from concourse.masks import make_identity

    FP32 = mybir.dt.float32
    BF16 = mybir.dt.bfloat16
    I16 = mybir.dt.int16
    Exp = mybir.ActivationFunctionType.Exp
    Mult = mybir.AluOpType.mult
    Add = mybir.AluOpType.add

    nc = bacc.Bacc(
        "TRN2", target_bir_lowering=False, debug=False, num_devices=N_CORES
    )

    xt_q_d = nc.dram_tensor("xt_q", [E, HALF], BF16, kind="ExternalInput").ap()
    xt_o_d = nc.dram_tensor("xt_o", [E, HALF], BF16, kind="ExternalInput").ap()
    wt_d = nc.dram_tensor("wt", [E, 320], BF16, kind="ExternalInput").ap()
    out_d = nc.dram_tensor("out", [D + 1, HALF], FP32, kind="ExternalOutput").ap()

    with tile.TileContext(nc) as tc, ExitStack() as ctx:
        const = ctx.enter_context(tc.tile_pool(name="const", bufs=1))
        big = ctx.enter_context(tc.tile_pool(name="big", bufs=1))
        # separate P pools per exp engine: a shared pool serializes the
        # scalar and vector exps against each other via slot-reuse ordering
        pps = ctx.enter_context(tc.tile_pool(name="pps", bufs=4))
        ppv = ctx.enter_context(tc.tile_pool(name="ppv", bufs=3))
        psA = ctx.enter_context(tc.tile_pool(name="psA", bufs=3, space="PSUM"))
        psB = ctx.enter_context(tc.tile_pool(name="psB", bufs=1, space="PSUM"))

        identB = const.tile([128, 128], BF16)
        junk = const.tile([128, 512], BF16)
        # memset on vector: ready right at the end of vector's preamble, so
        # the PE warm-up can start ~2.5us before make_identity's gpsimd
        # iota would allow.
        nc.vector.memset(junk[:, :], 0.0)
        make_identity(nc, identB)

        xt = big.tile([128, NE, S], BF16)  # x^T; cols [0, HALF) = own q-rows
        wt = big.tile([128, NE, 320], BF16)  # [WkT|WqT | WkT|WvT | WvT]
        qk = big.tile([128, HALF], BF16)  # rows 0-63 K^T own, 64-127 Q^T own
        kv = big.tile([128, HALF], BF16)  # rows 0-63 K^T oth, 64-127 V^T oth
        qts = big.tile([64, HALF], BF16)  # Q^T shifted to base partition 0
        vto = big.tile([64, HALF], BF16)  # V^T other shifted to base part. 0
        vt = big.tile([65, HALF], BF16)  # V^T own; row 64 = ones
        vn = big.tile([128, NKT, D + 1], BF16)  # V natural + ones column
        att_sb = big.tile([65, HALF], FP32)  # attn^T + denominator row

        # --- PE warm-up: the HAM duty controller halves the PE clock after
        # ~2.5us of idleness and takes 5-13us at half clock to restore full
        # duty, so keep the PE streaming junk matmuls from the instant its
        # preamble ends until the first input DMAs land.
        warm = psA.tile([128, 1024], FP32, tag="ps")
        for _ in range(N_WARM):
            nc.tensor.matmul(
                out=warm[0:128, 0:512],
                lhsT=junk[:, 0:128],
                rhs=junk[:, :],
                start=True,
                stop=True,
            )

        # --- input DMAs: ALL on the sync queue, issued in consumption
        # order. A single queue drains FIFO across all 16 DMA engines, so
        # arrival order == issue order and the PE never waits on a piece
        # that lost a queue-arbitration race (the old 3-queue split left
        # 12.6us of mid-ramp PE idle + HAM re-throttles). Measured stream
        # rate ~0.32 MB/us: wt ~9.5us, own 0:512 ~12.8, 512:1024 ~16.1,
        # own 1024:2048 ~22.7, oth chunks ~26/29/33/36us.
        xq_r = xt_q_d.rearrange("(t p) s -> p t s", p=128)
        xo_r = xt_o_d.rearrange("(t p) s -> p t s", p=128)
        nc.sync.dma_start(out=wt[:, :, :], in_=wt_d.rearrange("(t p) d -> p t d", p=128))
        # own cols 0:512 in two et-halves so A0's first matmuls start early
        nc.sync.dma_start(out=xt[:, 0:4, 0:512], in_=xq_r[:, 0:4, 0:512])
        nc.sync.dma_start(out=xt[:, 4:8, 0:512], in_=xq_r[:, 4:8, 0:512])
        nc.sync.dma_start(out=xt[:, :, 512:1024], in_=xq_r[:, :, 512:1024])
        nc.sync.dma_start(out=xt[:, :, 1024:2048], in_=xq_r[:, :, 1024:2048])
        for c in range(4):
            nc.sync.dma_start(
                out=xt[:, :, HALF + c * 512 : HALF + (c + 1) * 512],
                in_=xo_r[:, :, c * 512 : (c + 1) * 512],
            )

        nc.vector.memset(vt[64:65, :], 1.0)

        # one packed projection half-chunk of 512 cols
        def proj_half(w0, wm, dst, src_x0, d0):
            acc = psA.tile([128, 1024], FP32, tag="ps")
            for et in range(NE):
                nc.tensor.matmul(
                    out=acc[0:wm, 0:512],
                    lhsT=wt[:, et, w0 : w0 + wm],
                    rhs=xt[:, et, src_x0 : src_x0 + 512],
                    start=(et == 0),
                    stop=(et == NE - 1),
                )
            nc.vector.tensor_copy(out=dst[:, d0 : d0 + 512], in_=acc[0:wm, 0:512])

        def shift(dst, src, d0):
            # gpsimd's SWDGE queue is otherwise idle (inputs all ride the
            # sync queue now), so shifts never wait behind bulk input
            # pieces and don't steal ScalarE time from the exps.
            nc.gpsimd.dma_start(
                out=dst[:, d0 : d0 + 512], in_=src[64:128, d0 : d0 + 512]
            )

        def v_transpose(k):
            tp = psA.tile([128, 1024], BF16, tag="ps")
            if k < 16:  # own half: vt carries the ones row
                nc.tensor.transpose(
                    out=tp[0:128, 0:65],
                    in_=vt[:, k * 128 : (k + 1) * 128],
                    identity=identB[0:65, 0:65],
                )
                nc.vector.tensor_copy(out=vn[:, k, :], in_=tp[0:128, 0:65])
            else:  # other half: V^T shifted into vto (base partition 0)
                j = k - 16
                nc.tensor.transpose(
                    out=tp[0:128, 0:64],
                    in_=vto[:, j * 128 : (j + 1) * 128],
                    identity=identB[0:64, 0:64],
                )
                nc.vector.memset(vn[:, k, D : D + 1], 1.0)
                nc.vector.tensor_copy(out=vn[:, k, 0:D], in_=tp[0:128, 0:64])

        # --- prologue: A0 (needs own cols 0:512), C0 (same dep), A1 (own
        # 512:1024, which lands right as C0's matmuls finish) ---
        proj_half(0, 128, qk, 0, 0)
        shift(qts, qk, 0)
        proj_half(256, 64, vt[0:64, :], 0, 0)
        proj_half(0, 128, qk, 512, 512)
        shift(qts, qk, 512)

        # side-slot schedule for pass 0: iter k -> (kind, half-index).
        # Slots track the ordered-queue arrival times: own 1024:2048 by
        # ~iter 5, oth chunk c by ~iter 10+2c. Deadlines: A2/A3 feed
        # scores k=8/12, B-lump hh feeds scores k=16+4hh.
        SIDE = {
            1: ("C", 1),
            5: ("A", 2), 6: ("C", 2), 7: ("A", 3), 8: ("C", 3),
            10: ("B", 0), 12: ("B", 1), 14: ("B", 2), 16: ("B", 3),
        }

        def side_work(k):
            s = SIDE.get(k)
            if s is not None:
                kind, hh = s
                if kind == "A":
                    proj_half(0, 128, qk, hh * 512, hh * 512)
                    shift(qts, qk, hh * 512)
                elif kind == "C":
                    proj_half(256, 64, vt[0:64, :], hh * 512, hh * 512)
                else:
                    proj_half(128, 128, kv, HALF + hh * 512, hh * 512)
                    shift(vto, kv, hh * 512)
            if k == 2:
                v_transpose(0)
                v_transpose(1)
            elif k >= 3:
                v_transpose(k - 1)
                if k == NKT - 1:
                    v_transpose(NKT - 1)

        out_engs = [nc.sync, nc.gpsimd]

        # --- two q-passes of 1024 columns each ---
        for ps in range(2):
            att_ps = psB.tile([128, 1024], FP32)
            p_tiles = {}

            for k in range(NKT):
                if k < 16:
                    klhs = qk[0:64, k * 128 : (k + 1) * 128]
                else:
                    klhs = kv[0:64, (k - 16) * 128 : (k - 15) * 128]

                sc = psA.tile([128, 1024], FP32, tag="ps")
                for c in range(2):
                    q0 = ps * 1024 + c * 512
                    nc.tensor.matmul(
                        out=sc[:, c * 512 : (c + 1) * 512],
                        lhsT=klhs,
                        rhs=qts[:, q0 : q0 + 512],
                        start=True,
                        stop=True,
                    )
                eng = _exp_eng(k)
                if eng == "S":
                    p = pps.tile([128, 1024], BF16)
                    nc.scalar.activation(
                        out=p[:, :], in_=sc[:, :], func=Exp, scale=SCALE
                    )
                else:
                    p = ppv.tile([128, 1024], BF16)
                    nc.vector.tensor_scalar(
                        p[:, :].bitcast(I16), sc[:, :], A16, B16, Mult, Add
                    )
                p_tiles[k] = p

                if ps == 0:
                    side_work(k)
                # attn lags THREE iters behind scores: at lag 2 the exp
                # chain (ScalarE ~1.15us, 2 of every 3 tiles) misses the
                # deadline by ~230ns every iteration and paces the whole
                # pipeline at 1085ns/iter instead of the PE-bound 853ns.
                if k >= 3:
                    _attn(nc, att_ps, vn, p_tiles, k - 3)

            for kt in (NKT - 3, NKT - 2, NKT - 1):
                _attn(nc, att_ps, vn, p_tiles, kt)

            # ship this pass's attn^T + denominators (host normalizes)
            for c in range(2):
                cols = slice(ps * 1024 + c * 512, ps * 1024 + (c + 1) * 512)
                pcols = slice(c * 512, (c + 1) * 512)
                nc.vector.tensor_copy(out=att_sb[:, cols], in_=att_ps[0:65, pcols])
                out_engs[c].dma_start(out=out_d[:, cols], in_=att_sb[:, cols])

    nc.compile()
    _CACHE["nc"] = nc
    return nc


def _attn(nc, att_ps, vn, p_tiles, k):
    p = p_tiles.pop(k)
    for c in range(2):
        nc.tensor.matmul(
            out=att_ps[0:65, c * 512 : (c + 1) * 512],
            lhsT=vn[:, k, :],
            rhs=p[:, c * 512 : (c + 1) * 512],
            start=(k == 0),
            stop=(k == NKT - 1),
            skip_group_check=True,
        )


def _make_in_maps(x, Wq, Wk, Wv):
    import ml_dtypes

    bf16 = ml_dtypes.bfloat16
    xT = np.ascontiguousarray(x.transpose(0, 2, 1)).astype(bf16)  # [B, E, S]
    wt = np.concatenate(
        [Wk.T, Wq.T, Wk.T, Wv.T, Wv.T], axis=1
    ).astype(bf16)  # [E, 320]
    in_maps = []
    for c in range(N_CORES):
        b, h = divmod(c, 2)
        in_maps.append(
            {
                "xt_q": np.ascontiguousarray(xT[b, :, h * HALF : (h + 1) * HALF]),
                "xt_o": np.ascontiguousarray(
                    xT[b, :, (1 - h) * HALF : (2 - h) * HALF]
                ),
                "wt": wt,
            }
        )
    return in_maps


def _run(x, Wq, Wk, Wv, trace=False):
    from concourse.bass_utils import run_bass_kernel_spmd

    nc = _build()
    in_maps = _make_in_maps(x, Wq, Wk, Wv)
    res = run_bass_kernel_spmd(
        nc, in_maps, core_ids=list(range(N_CORES)), trace=trace
    )
    out = np.empty((B, S, D), dtype=np.float32)
    for c in range(N_CORES):
        b, h = divmod(c, 2)
        att = res.results[c]["out"]  # [65, HALF]: attn^T rows + denom row
        out[b, h * HALF : (h + 1) * HALF, :] = (att[0:D] / att[D : D + 1]).T
    return out, res


def kernel(x, Wq, Wk, Wv):
    out, _ = _run(
        np.asarray(x, dtype=np.float32),
        np.asarray(Wq, dtype=np.float32),
        np.asarray(Wk, dtype=np.float32),
        np.asarray(Wv, dtype=np.float32),
    )
    return out

